# revision 40
# baseline (speedup 1.0000x reference)
"""Multi-head causal self-attention (B=64, T=256, C=384, H=6) on 8 NeuronCores.

Data-parallel over batch: each core processes 8 batches (2048 tokens).
Layouts avoid any device-side transposes:
  - xT, Q.T, K.T feature-major [C, tokens]; V token-major [tokens, C]
  - scores computed transposed (S.T[tk, tq]) so exp(S.T) feeds P.T@V directly
  - attention output lands feature-major (catT) for the output projection

vs. 85us baseline (now ~74-76us):
  - Q/K projections run in fp8 (e4m3) with DoubleRow perf mode: contraction
    C=384 becomes one K=256 DR pass + one K=128 normal fp8 pass (2 column
    streams instead of 3). fp8 on the Q/K path only is numerically safe: the
    C**-0.5 softmax scale damps logit quantization (~6e-3 total vs 2e-2 gate);
    fp8 on V/proj fails the gate (measured 3-5e-2), so those stay bf16.
    x is shipped twice (fp8 for Q/K, bf16 for V's stationary).
  - score matmuls emitted h0/h1 adjacent per column-count so the two head
    row-groups stream concurrently; O and Z matmuls reordered (h0a, h1a,
    h0b, h1b) so the two head col-groups overlap (measured dt ~3ns)
  - causal mask as two gpsimd affine_selects grouped ACROSS heads per diag
    block (d0-both-heads first): all round-a O/Z matmuls gate on one op, so
    both col-groups release together (per-head selects skewed h1 by ~450ns)
  - PSUM: pa bufs=3 (qk's DR matmuls produce a tile per 426ns but its drain
    costs ~650ns; 2 bufs made qk drain-rate-bound), poz bufs=1
  - drains: q on vector, k on scalar (parallel, DIFFERENT banks -- same-bank
    concurrent ScE+DVE PSUM access is a HW hazard; a cross-engine col-split
    drain produced a NaN run), each as two sequential 256-col ops so even
    batches' scores release early; v-copies vector, y-adds scalar + co2 vec
  - exp split in two scalar ops, diagonal blocks first: the gpsimd mask and
    the O/Z matmuls start ~550ns earlier per head-pair (shortens the
    latency-bound tail chains)
  - qk emitted one tile ahead; proj(t) of the last tile split in halves
    around the final attention block so the tail stays dense
  - input DMAs batched into few triggers (each costs ~640ns of its engine),
    first-use-ordered: wq8+xq(t0) fp8, then x(t0) bf16 + wvo, then the rest;
    y DMAs on the idle gpsimd/sync rings, last tile on all three
  - warm-up: 20 N=256 matmuls on a memset tile (DMA rings only start moving
    data ~9us; vector can memset at ~6.2us) so HAM hits 8/8 before real work

HW constraints learned the hard way:
  - concurrent row-packed matmuls (both heads, M=128) must write DIFFERENT
    PSUM banks -- same-bank same-partition concurrent PE writes abort.
    Column-packed matmuls (O.T/Z) write disjoint partitions, may share.
  - ScalarE + VectorE may touch PSUM in parallel only on different banks.
  - within one PSUM bank+partition range, an accumulation group must fully
    finish (stop=True) before another group's start=True touches it (the
    start clears has_written for the whole bank in those partitions).
"""

import sys

import ml_dtypes
import numpy as np

for _p in ("/opt/trn_rl_repo", "/root/.axon_site/_ro/trn_rl_repo"):
    if _p not in sys.path:
        sys.path.insert(0, _p)

import concourse.bass as bass
import concourse.tile as tile
from concourse import bacc, mybir
from concourse.bass_utils import run_bass_kernel_spmd

B, T, C, H, D = 64, 256, 384, 6, 64
NCORES = 8
BB = B // NCORES  # batches per core = 8
TOK = BB * T      # tokens per core = 2048
SCALE = float(C) ** -0.5
F32 = mybir.dt.float32
BF16 = mybir.dt.bfloat16
F8 = mybir.dt.float8e4
NPBF = ml_dtypes.bfloat16
NPF8 = ml_dtypes.float8_e4m3
DR = mybir.MatmulPerfMode.DoubleRow

NT4 = TOK // 512  # 4 column-chunks of 512 tokens
NKC = C // 128    # 3 chunks of 128 over feature dim
NWARM = 20


def build_module():
    nc = bacc.Bacc("TRN2", target_bir_lowering=False, debug=False)

    xq8 = nc.dram_tensor("xq8", [C, TOK], F8, kind="ExternalInput").ap()
    wq8 = nc.dram_tensor("wq8", [C, 2 * C], F8, kind="ExternalInput").ap()
    xT = nc.dram_tensor("xT", [C, TOK], BF16, kind="ExternalInput").ap()
    wvo = nc.dram_tensor("wvo", [C, 2 * C], BF16, kind="ExternalInput").ap()
    wobc = nc.dram_tensor("wobc", [C, 1], F32, kind="ExternalInput").ap()
    ones = nc.dram_tensor("ones", [128, 64], BF16, kind="ExternalInput").ap()
    yT = nc.dram_tensor("yT", [C, TOK], BF16, kind="ExternalOutput").ap()

    with tile.TileContext(nc) as tc:
        import contextlib

        ctx = contextlib.ExitStack()
        with ctx:
            consts = ctx.enter_context(tc.tile_pool(name="consts", bufs=1))

            def ptile(name, shape, dt=BF16):
                return consts.tile(shape, dt, tag=name, name=name)

            # fp8 Q/K weights: [128, (kc, [wq|wk])] -- kc planes side by side
            # so the DoubleRow middle dim strides between kc0 and kc1.
            wq8_sb = ptile("wq8", [128, NKC * 2 * C], F8)
            wq8v = wq8_sb.rearrange("p (kc x) -> p kc x", x=2 * C)
            # fp8 x: one tile, [128, (kc, TOK)] so a whole kc plane is one DMA
            xq_sb = ptile("xq", [128, NKC * TOK], F8)
            xqv = xq_sb.rearrange("p (kc n) -> p kc n", n=TOK)

            wvo_sb = [ptile(f"wvo{k}", [128, 2 * C]) for k in range(NKC)]
            wv_sb = [w[:, 0:C] for w in wvo_sb]
            wo_sb = [w[:, C:2 * C] for w in wvo_sb]
            wob_sb = [ptile(f"wob{k}", [128, 1], F32) for k in range(NKC)]
            ones_sb = ptile("ones", [128, 64])
            xtf_sb = [ptile(f"xt{k}", [128, TOK]) for k in range(NKC)]
            xt_sb = [[xtf_sb[k][:, 512 * t:512 * (t + 1)] for t in range(NT4)] for k in range(NKC)]
            qt_sb = [[ptile(f"qt{k}_{t}", [128, 512]) for t in range(NT4)] for k in range(NKC)]
            kt_sb = [[ptile(f"kt{k}_{t}", [128, 512]) for t in range(NT4)] for k in range(NKC)]
            cat_sb = [[ptile(f"cat{k}_{t}", [128, 512]) for t in range(NT4)] for k in range(NKC)]
            v_sb = [ptile(f"v{t}", [128, C]) for t in range(2 * BB)]  # 16 token-blocks of 128

            # ---- input DMAs. Three ~80GB/s rings (gpsimd/scalar/sync),
            # k-row-split so each piece lands via all three in parallel,
            # batched into few triggers (each trigger costs ~640ns of the
            # issuing engine). Order by first use: fp8 wq8 + xq(t0) (first
            # qk matmuls), bf16 x(t0) + wvo (vproj 0), then the rest.
            # NOTE: finer-grained splits measured WORSE: Tile's DMA flow
            # control makes each trigger wait for the ring's previous
            # transfer, and those waits head-of-line-block the scalar queue
            # ahead of its first k-drains.
            ring = [nc.gpsimd, nc.scalar, nc.sync]
            nc.sync.dma_start(out=ones_sb, in_=ones)
            for k in range(NKC):
                ring[k].dma_start(
                    out=wq8_sb[:, 2 * C * k:2 * C * (k + 1)],
                    in_=wq8[128 * k:128 * (k + 1), :])
            for k in range(NKC):
                ring[k].dma_start(
                    out=xqv[:, k, 0:512],
                    in_=xq8[128 * k:128 * (k + 1), 0:512])
            for k in range(NKC):
                ring[k].dma_start(
                    out=xt_sb[k][0],
                    in_=xT[128 * k:128 * (k + 1), 0:512])
            for k in range(NKC):
                ring[k].dma_start(
                    out=wvo_sb[k],
                    in_=wvo[128 * k:128 * (k + 1), :])
            for k in range(NKC):
                ring[k].dma_start(
                    out=xqv[:, k, 512:TOK],
                    in_=xq8[128 * k:128 * (k + 1), 512:TOK])
            for k in range(NKC):
                ring[k].dma_start(
                    out=xtf_sb[k][:, 512:TOK],
                    in_=xT[128 * k:128 * (k + 1), 512:TOK])
            for k in range(NKC):
                nc.sync.dma_start(out=wob_sb[k], in_=wobc[128 * k:128 * (k + 1), :])

            # ---- PSUM pools: pa 2x1 + ps 2x2 + poz 2x1 = 8 banks ----
            pa = ctx.enter_context(tc.tile_pool(name="pa", bufs=3, space="PSUM"))
            ps = ctx.enter_context(tc.tile_pool(name="ps", bufs=2, space="PSUM"))
            poz = ctx.enter_context(tc.tile_pool(name="poz", bufs=1, space="PSUM"))

            pt_pool = ctx.enter_context(tc.tile_pool(name="ptp", bufs=BB * H // 2))
            rp_pool = ctx.enter_context(tc.tile_pool(name="rpp", bufs=6))
            y_pool = ctx.enter_context(tc.tile_pool(name="yp", bufs=4))

            def qk(t):
                """Q.T / K.T = W @ x.T for 512-token tile t, fp8 DoubleRow:
                one K=256 DR pass (kc0+kc1) + one K=128 fp8 pass (kc2)."""
                for co in range(NKC):
                    for wi, out_sb in ((0, qt_sb), (1, kt_sb)):
                        pqk = pa.tile([128, 512], F32, tag="pa", name=f"p{wi}{co}_{t}")
                        c0 = C * wi + 128 * co
                        nc.tensor.matmul(
                            pqk,
                            wq8v[:, 0:2, c0:c0 + 128],
                            xqv[:, 0:2, 512 * t:512 * (t + 1)],
                            start=True, stop=False,
                            perf_mode=DR,
                        )
                        nc.tensor.matmul(
                            pqk,
                            wq8v[:, 2, c0:c0 + 128],
                            xqv[:, 2, 512 * t:512 * (t + 1)],
                            start=False, stop=True,
                        )
                        # q drains on vector, k drains on scalar: the two
                        # engines work different PSUM banks in parallel
                        # (same-bank concurrent ScE+DVE access is a HW
                        # hazard). Each drain is two sequential 256-col ops
                        # on its engine: even batches' scores only need
                        # cols 0:256, so they release ~300ns earlier
                        if wi == 0:
                            nc.vector.tensor_copy(out_sb[co][t][:, 0:256], pqk[:, 0:256])
                            nc.vector.tensor_copy(out_sb[co][t][:, 256:512], pqk[:, 256:512])
                        else:
                            nc.scalar.copy(out_sb[co][t][:, 0:256], pqk[:, 0:256])
                            nc.scalar.copy(out_sb[co][t][:, 256:512], pqk[:, 256:512])

            def vproj(t):
                """V token-major [tok, C] for the 4 token-blocks of tile t
                (bf16: fp8 on the V path fails the accuracy gate)."""
                for j in range(4):
                    tb = 4 * t + j
                    pv = pa.tile([128, C], F32, tag="pa", name=f"pv{tb}")
                    for kc in range(NKC):
                        nc.tensor.matmul(
                            pv,
                            xt_sb[kc][t][:, 128 * j:128 * (j + 1)],
                            wv_sb[kc],
                            start=(kc == 0),
                            stop=(kc == NKC - 1),
                        )
                    nc.vector.tensor_copy(v_sb[tb], pv)

            def attn(b, hps=None):
                """Attention for batch b, head pairs. Score bank layout per
                head (bank hh = p_s cols 512*hh):
                  cols 0:128   d0 = diag block (tk-blk0 x tq 0:128)
                  cols 128:256 d1 = diag block (tk-blk1 x tq 128:256)
                  cols 256:384 f  = full block (tk-blk0 x tq 128:256)
                pt packs both heads: [d0h0|d1h0|d0h1|d1h1|fh0|fh1] so the 4
                diag blocks are contiguous at stride 128 -> ONE 3D gpsimd
                affine_select masks both heads (a single producer, so both
                heads' O matmuls become ready simultaneously; two serial
                selects skewed the second head by ~430ns)."""
                t4b, qc = b // 2, (b % 2) * 256
                for hp in (range(H // 2) if hps is None else hps):
                    kt = kt_sb[hp][t4b]
                    qt = qt_sb[hp][t4b]
                    # scores: head hh lives entirely in PSUM bank hh of the
                    # tile. Emit the two heads' same-width matmuls adjacently
                    # so the two row-groups stream concurrently.
                    p_s = ps.tile([128, 1024], F32, tag="ps", name=f"s{b}_{hp}")
                    for hh in range(2):
                        r0, sb = 64 * hh, 512 * hh
                        nc.tensor.matmul(
                            p_s[:, sb:sb + 256],
                            kt[r0:r0 + 64, qc:qc + 128],
                            qt[r0:r0 + 64, qc:qc + 256],
                            start=True, stop=True,
                        )
                    for hh in range(2):
                        r0, sb = 64 * hh, 512 * hh
                        nc.tensor.matmul(
                            p_s[:, sb + 256:sb + 384],
                            kt[r0:r0 + 64, qc + 128:qc + 256],
                            qt[r0:r0 + 64, qc + 128:qc + 256],
                            start=True, stop=True,
                        )
                    # P.T = exp(S.T / sqrt(C)), d0 blocks first so the
                    # gpsimd mask (and then the O/Z matmuls) can start
                    # ~550ns earlier; f+d1 exp overlaps the mask
                    pt = pt_pool.tile([128, 768], BF16, tag="pt", name=f"pt{b}_{hp}")
                    nc.scalar.activation(
                        pt.rearrange("p (c i) -> p c i", i=128)[:, 0::3, :],
                        p_s.rearrange("p (c i) -> p c i", i=128)[:, 0::4, :],
                        mybir.ActivationFunctionType.Exp, scale=SCALE,
                    )
                    nc.scalar.activation(
                        pt.rearrange("p (a q) -> p a q", q=384)[:, :, 128:384],
                        p_s.rearrange("p (a q) -> p a q", q=512)[:, :, 128:384],
                        mybir.ActivationFunctionType.Exp, scale=SCALE,
                    )
                    # causal mask, grouped ACROSS heads per diag block: the
                    # first select covers both heads' d0 blocks (all the
                    # round-a O/Z matmuls gate on this one op, so both
                    # col-groups become ready together -- two per-head
                    # selects skewed head 1 by ~450ns), the second select
                    # covers both d1 blocks and only gates round b.
                    ptc = pt.rearrange("p (c i) -> p c i", i=128)
                    for cblk in (0, 2):
                        sel = ptc[:, cblk::3, :]
                        nc.gpsimd.affine_select(
                            out=sel, in_=sel,
                            pattern=[[0, 2], [1, 128]],
                            compare_op=mybir.AluOpType.is_ge,
                            fill=0.0, base=0, channel_multiplier=-1,
                        )
                    rhs_a = [pt[:, 0:256], pt[:, 384:640]]
                    rhs_b = [pt[:, 256:384], pt[:, 640:768]]
                    # O.T (cols 0:256) and broadcast Z (cols 256:512) in one
                    # bank; col-packed writes are partition-disjoint -> safe.
                    # Emit (h0,h1) pairs adjacently per accumulate-round so
                    # the two col-groups overlap.
                    po = poz.tile([128, 512], F32, tag="poz", name=f"poz{b}_{hp}")
                    for hh in range(2):
                        h = 2 * hp + hh
                        r0 = 64 * hh
                        nc.tensor.matmul(
                            po[r0:r0 + 64, 0:256],
                            v_sb[2 * b][:, 64 * h:64 * (h + 1)],
                            rhs_a[hh],
                            start=True, stop=False,
                            tile_position=(0, r0), skip_group_check=True,
                        )
                    for hh in range(2):
                        h = 2 * hp + hh
                        r0 = 64 * hh
                        nc.tensor.matmul(
                            po[r0:r0 + 64, 128:256],
                            v_sb[2 * b + 1][:, 64 * h:64 * (h + 1)],
                            rhs_b[hh],
                            start=False, stop=True,
                            tile_position=(0, r0), skip_group_check=True,
                        )
                    for hh in range(2):
                        r0 = 64 * hh
                        nc.tensor.matmul(
                            po[r0:r0 + 64, 256:512],
                            ones_sb, rhs_a[hh],
                            start=True, stop=False,
                            tile_position=(0, r0), skip_group_check=True,
                        )
                    for hh in range(2):
                        r0 = 64 * hh
                        nc.tensor.matmul(
                            po[r0:r0 + 64, 384:512],
                            ones_sb, rhs_b[hh],
                            start=False, stop=True,
                            tile_position=(0, r0), skip_group_check=True,
                        )
                    # normalize: cat = O.T * (1/Z). Safe to read only the Z
                    # half: the PE completes matmuls in program order, so by
                    # the time the Z writes are done the O writes are too.
                    rp = rp_pool.tile([128, 256], F32, tag="rp", name=f"rp{b}_{hp}")
                    nc.vector.reciprocal_approx_fast(rp, po[:, 256:512])
                    nc.vector.tensor_mul(
                        cat_sb[hp][t4b][:, qc:qc + 256], po[:, 0:256], rp,
                    )

            def proj(t, half=None):
                """y.T = Wo @ catT + bo for tile t (or one 256-token half of
                it), stored bf16. y-adds alternate scalar/vector; mid-kernel
                y DMAs go out on the idle sync/gpsimd rings (scalar's trigger
                time is precious), the last tile uses all three rings."""
                last = t == NT4 - 1
                c0 = 512 * t if half is None else 512 * t + 256 * half
                nw = 512 if half is None else 256
                s0 = c0 - 512 * t
                for co in range(NKC):
                    pyk = pa.tile([128, nw], F32, tag="pa", name=f"py{co}_{c0}")
                    for kc in range(NKC):
                        nc.tensor.matmul(
                            pyk,
                            wo_sb[kc][:, 128 * co:128 * (co + 1)],
                            cat_sb[kc][t][:, s0:s0 + nw],
                            start=(kc == 0),
                            stop=(kc == NKC - 1),
                        )
                    yt = y_pool.tile([128, nw], BF16, tag="yt", name=f"yt{co}_{c0}")
                    if co == 2:
                        nc.vector.tensor_scalar_add(yt, pyk, wob_sb[co][:, 0:1])
                    else:
                        nc.scalar.add(yt, pyk, wob_sb[co][:, 0:1])
                    oring = ring[co] if last else (nc.gpsimd, nc.sync, nc.sync)[co]
                    oring.dma_start(
                        out=yT[128 * co:128 * (co + 1), c0:c0 + nw],
                        in_=yt,
                    )

            # ---- PE warm-up: dummy matmuls on a memset tile. The DMA
            # rings don't move data until ~9us, but vector can memset at
            # ~6.2us; N=256 matmuls until ~11.5us keep the PE busy through
            # a full HAM window so the real matmuls start at 8/8 clock ----
            wtile = consts.tile([128, 256], BF16, tag="warm", name="warm")
            nc.vector.memset(wtile, 0.0)
            wz = pa.tile([128, 256], F32, tag="pa", name="warmz")
            for i in range(NWARM):
                nc.tensor.matmul(wz, wtile[:, 0:128], wtile, start=True, stop=True)

            qk(0)
            for t in range(NT4):
                vproj(t)
                if t < NT4 - 1:
                    attn(2 * t)
                    # proj(t-1) after attn(2t): its matmuls wait on the
                    # previous iteration's cat normalize, and emitting them
                    # first would block ready attention matmuls
                    if t > 0:
                        proj(t - 1)
                    # qk one tile ahead: its matmuls and drains fill the
                    # attention latency bubbles, and its qt/kt are fully
                    # drained well before attn(2t+2) needs them
                    qk(t + 1)
                    attn(2 * t + 1)
                else:
                    # last tile: interleave the two batches' attention
                    # head-pair-wise so the latency-bound tail has two
                    # independent chains to overlap instead of one
                    attn(2 * t, hps=[0])
                    attn(2 * t + 1, hps=[0])
                    attn(2 * t, hps=[1])
                    proj(t - 1)
                    attn(2 * t + 1, hps=[1])
                    attn(2 * t, hps=[2])
                    proj(t, half=0)
                    attn(2 * t + 1, hps=[2])
            proj(NT4 - 1, half=1)

    nc.compile()
    return nc


def make_in_maps(x, Wk, Wq, Wv, Wo, bo):
    x = np.asarray(x, np.float32)
    wq8c = np.concatenate(
        [np.asarray(w, np.float32).T for w in (Wq, Wk)], axis=1
    ).astype(NPF8)
    wvoc = np.concatenate(
        [np.asarray(w, np.float32).T for w in (Wv, Wo)], axis=1
    ).astype(NPBF)
    wobc = np.ascontiguousarray(np.asarray(bo, np.float32).reshape(C, 1))
    ones = np.ones((128, 64), NPBF)
    in_maps = []
    for i in range(NCORES):
        xi = x[BB * i:BB * (i + 1)].reshape(TOK, C)
        xiT = np.ascontiguousarray(xi.T)
        in_maps.append({
            "xq8": xiT.astype(NPF8),
            "wq8": wq8c,
            "xT": xiT.astype(NPBF),
            "wvo": wvoc,
            "wobc": wobc,
            "ones": ones,
        })
    return in_maps


_NC_CACHE = None


def kernel(x, Wk, Wq, Wv, Wo, bo):
    global _NC_CACHE
    if _NC_CACHE is None:
        _NC_CACHE = build_module()
    nc = _NC_CACHE
    in_maps = make_in_maps(x, Wk, Wq, Wv, Wo, bo)
    res = run_bass_kernel_spmd(nc, in_maps, core_ids=list(range(NCORES)))
    outs = []
    for i in range(NCORES):
        yt = np.asarray(res.results[i]["yT"]).astype(np.float32)
        outs.append(yt.T.reshape(BB, T, C))
    return np.concatenate(outs, axis=0).astype(np.float32)
